# revision 13
# baseline (speedup 1.0000x reference)
"""Trainium2 Bass kernel for nn_DualAxisAggAttn (dual-axis aggregation attention).

Reference semantics per batch image x[C=256, H=64, W=64], twice (W axis then H axis):
  qkv = conv1x1(x) -> {q:[1], k:[C], v:[C]};  s = softmax_axis(q)
  ctx[c,a] = sum_r k*s;  out = x + sigmoid(v) * ctx_bcast;  y = conv1x1(out)

Distribution: data-parallel over batch (16 images -> 2 per NeuronCore x 8 cores).

Structure (v4):
  - STAGE COLLAPSE: every stage-H op on x_w = WfW @ out_W is linear in
    channels, so WfW is folded into stage-H's weights on the HOST
    (qH' = qH@WfW, WvH' = WvH@WfW, WkH' = WkH@WfW, F2 = WfH@WfW). The
    stage-W fusion conv never runs on device; out_W is materialized once
    (O = x + g2) and feeds all H matmuls. Removes 2 of 8 big matmul
    passes AND the stage-W PSUM->SBUF eviction on the ACT engine.
  - stage-W qkv matmuls in fp8e4 DoubleRow (full 256-contraction per
    pass, 2x PE): only softmax weights E and the gate see the ~4%
    quantization, both of which are insensitive paths.
  - key-path linearity: ctx = Wk @ (sum_r x*E) / S (N=4096 -> 64).
  - q row replicated 128x in its m-tile -> exp(q) lands partition-broadcast.
  - sigmoid via tanh: {exp, tanh, copy} share ONE ACT table set
    (AF.Sigmoid does not -- a table swap costs 1.3us).
  - DVE has ~300ns/op overhead at ~0.38ns/elem/p streaming: elementwise
    ops (u, gate+1, g2, O) run on 1024-column chunk PAIRS; E lives in
    slot 0 of the [128, 3, HW] u tile so E+u reduce in ONE merged tree.
  - y stored bf16 on device, upcast to f32 on host (halves y DMA).
"""

import numpy as np
import ml_dtypes
from contextlib import ExitStack

import concourse.bass as bass
import concourse.bacc as bacc
import concourse.tile as tile
import concourse.mybir as mybir
from concourse.bass_utils import run_bass_kernel_spmd

F32 = mybir.dt.float32
BF16 = mybir.dt.bfloat16
FP8 = mybir.dt.float8e4
AF = mybir.ActivationFunctionType
ALU = mybir.AluOpType
AX = mybir.AxisListType
PM = mybir.MatmulPerfMode
NPBF = ml_dtypes.bfloat16
NPF8 = ml_dtypes.float8_e4m3fn

B, C, H, W = 16, 256, 64, 64
HW = H * W
NCORES = 8
BPC = B // NCORES
KT = 2
CH = 512
NCH = HW // CH
GRP = CH // 64
CP = 2 * CH          # DVE pair width
NP_ = HW // CP       # pairs per stage-batch

_BUILD_CACHE = {}
LAST_RESULTS = None


class _Stage:
    """One attention stage for one batch: p1 (qkv+u), p2 (reduce+ctx), p3."""

    def __init__(self, nc, pools, axis_w, src, stat, wk, bias, src8=None):
        self.nc, self.axis_w = nc, axis_w
        self.src, self.stat, self.wk, self.bias = src, stat, wk, bias
        self.src8 = src8  # callable j -> fp8 [128, KT, CH] chunk (W stage only)
        (self.pbig, self.pgate, self.pchunk, self.pctx, self.pq, self.pv,
         self.phv) = pools

    def p1_alloc(self):
        # slot 0 = E (exp(q)), slots 1:3 = u = x*E  -> one merged reduce tree
        self.u = self.pbig.tile([128, 3, HW], BF16, tag="u")
        self.gate = self.pgate.tile([128, 2, HW], BF16, tag="gate")

    def p1_chunk(self, j):
        nc, stat, bias = self.nc, self.stat, self.bias
        sl = bass.ts(j, CH)
        bv2 = bias.get("bv2")
        ps_q = self.pq.tile([128, CH], F32, tag="q")
        ps_v = self.pv.tile([128, 2 * CH], F32, tag="vf")
        if self.src8 is not None:
            x8 = self.src8(j)
            nc.tensor.matmul(ps_q[:], stat[:, :, 2, :], x8[:], perf_mode=PM.DoubleRow)
            nc.tensor.matmul(ps_v[:, 0:CH], stat[:, :, 0, :], x8[:], perf_mode=PM.DoubleRow)
            nc.tensor.matmul(ps_v[:, CH:], stat[:, :, 1, :], x8[:], perf_mode=PM.DoubleRow)
        else:
            for kt in range(KT):
                st, sp = kt == 0, kt == KT - 1
                rhs = self.src[:, kt, sl]
                nc.tensor.matmul(ps_q[:], stat[:, kt, 2, :], rhs, start=st, stop=sp)
                nc.tensor.matmul(ps_v[:, 0:CH], stat[:, kt, 0, :], rhs, start=st, stop=sp)
                nc.tensor.matmul(ps_v[:, CH:], stat[:, kt, 1, :], rhs, start=st, stop=sp)
        nc.scalar.activation(self.u[:, 0, sl], ps_q[:], AF.Exp, bias=bias["zb"])
        if bv2 is None:
            nc.scalar.activation(
                self.gate[:, :, sl], ps_v[:].rearrange("p (c n) -> p c n", c=2),
                AF.Tanh, scale=0.5,
            )
        else:
            nc.scalar.activation(self.gate[:, 0, sl], ps_v[:, 0:CH], AF.Tanh, bias=bv2[0], scale=0.5)
            nc.scalar.activation(self.gate[:, 1, sl], ps_v[:, CH:], AF.Tanh, bias=bv2[1], scale=0.5)

    def p1_pair(self, jp):
        """u = x*E and gate += 1 over a 1024-column chunk pair."""
        nc = self.nc
        slp = bass.ts(jp, CP)
        eb = self.u[:, 0, slp].unsqueeze(1).broadcast_to([128, 2, CP])
        nc.vector.tensor_tensor(self.u[:, 1:3, slp], self.src[:, :, slp], eb, op=ALU.mult)
        nc.vector.tensor_scalar_add(self.gate[:, :, slp], self.gate[:, :, slp], 1.0)

    def p2(self):
        nc, pctx, phv = self.nc, self.pctx, self.phv
        t3 = phv.tile([128, 3, 2048], BF16, tag="t3")
        SX = pctx.tile([128, 3, 64], F32, tag="SX")
        if self.axis_w:
            # reduce over w (inner 64): [3, 64 groups, r] halving + TR
            v4 = self.u[:].rearrange("p c (a r) -> p c a r", r=64)
            hv = t3[:].rearrange("p c (a r) -> p c a r", r=32)
            nc.vector.tensor_tensor(hv[:, :, :, :], v4[:, :, :, 0:32], v4[:, :, :, 32:64], op=ALU.add)
            nc.vector.tensor_tensor(hv[:, :, :, 0:16], hv[:, :, :, 0:16], hv[:, :, :, 16:32], op=ALU.add)
            nc.vector.tensor_tensor(hv[:, :, :, 0:8], hv[:, :, :, 0:8], hv[:, :, :, 8:16], op=ALU.add)
            nc.vector.tensor_reduce(SX[:], hv[:, :, :, 0:8], axis=AX.X, op=ALU.add)
        else:
            # reduce over h (outer): contiguous halving tree on [3, n]
            nc.vector.tensor_tensor(t3[:, :, :], self.u[:, :, 0:2048], self.u[:, :, 2048:4096], op=ALU.add)
            n = 1024
            while n >= 128:
                nc.vector.tensor_tensor(t3[:, :, 0:n], t3[:, :, 0:n], t3[:, :, n:2 * n], op=ALU.add)
                n //= 2
            nc.vector.tensor_tensor(SX[:], t3[:, :, 0:64], t3[:, :, 64:128], op=ALU.add)

        R = pctx.tile([128, 64], F32, tag="R")
        nc.vector.reciprocal(R[:], SX[:, 0, :])
        xn = pctx.tile([128, 2, 64], BF16, tag="xn")
        rb = R[:].unsqueeze(1).broadcast_to([128, 2, 64])
        nc.vector.tensor_tensor(xn[:], SX[:, 1:3, :], rb, op=ALU.mult)

        bk2 = self.bias.get("bk2")
        ctx_t = pctx.tile([128, 2, 64], BF16, tag="ctx")
        for mt in range(2):
            ps_c = self.pq.tile([128, 64], F32, tag="q")
            for ct in range(2):
                nc.tensor.matmul(ps_c[:], self.wk[:, ct, mt, :], xn[:, ct, :],
                                 start=ct == 0, stop=ct == 1)
            if bk2 is None:
                nc.vector.tensor_scalar_mul(ctx_t[:, mt, :], ps_c[:], 0.5)
            else:
                nc.vector.tensor_scalar(ctx_t[:, mt, :], ps_c[:], 0.5, bk2[mt],
                                        op0=ALU.mult, op1=ALU.add)
        self.ctx_t = ctx_t

    def g2_pair(self, jp):
        """g2 = gate1 * ctx_broadcast for a chunk pair -> [128, 2, 2*GRP, 64]."""
        nc = self.nc
        g2 = self.pchunk.tile([128, 2, 2 * GRP, 64], BF16, tag="g2")
        gv = self.gate[:, :, bass.ts(jp, CP)].rearrange("p c (a r) -> p c a r", r=64)
        if self.axis_w:
            cb = self.ctx_t[:, :, bass.ts(jp, 2 * GRP)].unsqueeze(3).broadcast_to([128, 2, 2 * GRP, 64])
        else:
            cb = self.ctx_t[:].unsqueeze(2).broadcast_to([128, 2, 2 * GRP, 64])
        nc.vector.tensor_tensor(g2[:], gv, cb, op=ALU.mult)
        return g2


def _build(flags):
    bvW0, bkW0, bvH0, bkH0, byH0 = flags
    nc = bacc.Bacc(trn_type="TRN2", target_bir_lowering=False, debug=False)

    x_d = nc.dram_tensor("x", [BPC, C, HW], BF16, kind="ExternalInput").ap()
    x8_d = nc.dram_tensor("x8", [BPC, C, HW], FP8, kind="ExternalInput").ap()
    statW_d = nc.dram_tensor("statW", [128, KT, 3, 128], FP8, kind="ExternalInput").ap()
    statH_d = nc.dram_tensor("statH", [128, KT, 3, 128], BF16, kind="ExternalInput").ap()
    wkW_d = nc.dram_tensor("wkW", [128, KT, 2, 128], BF16, kind="ExternalInput").ap()
    wkH_d = nc.dram_tensor("wkH", [128, KT, 2, 128], BF16, kind="ExternalInput").ap()
    f2_d = nc.dram_tensor("f2", [128, KT, 2, 128], BF16, kind="ExternalInput").ap()
    fg_d = nc.dram_tensor("fg", [128, KT, 2, 128], BF16, kind="ExternalInput").ap()
    bias_d = nc.dram_tensor("biases", [5, 2, 128], F32, kind="ExternalInput").ap()
    y_d = nc.dram_tensor("y", [BPC, C, HW], BF16, kind="ExternalOutput").ap()

    with tile.TileContext(nc) as tc, ExitStack() as ctx:
        wp = ctx.enter_context(tc.tile_pool(name="weights", bufs=1))
        xp = ctx.enter_context(tc.tile_pool(name="x", bufs=2))
        x8p = ctx.enter_context(tc.tile_pool(name="x8", bufs=3))
        op_ = ctx.enter_context(tc.tile_pool(name="O", bufs=2))
        pbig = ctx.enter_context(tc.tile_pool(name="big", bufs=2))
        pgate = ctx.enter_context(tc.tile_pool(name="gate", bufs=3))
        pchunk = ctx.enter_context(tc.tile_pool(name="chunk", bufs=2))
        pctx = ctx.enter_context(tc.tile_pool(name="ctx", bufs=3))
        phv = ctx.enter_context(tc.tile_pool(name="hv", bufs=1))
        yp = ctx.enter_context(tc.tile_pool(name="yev", bufs=2))
        pq = ctx.enter_context(tc.tile_pool(name="psq", bufs=2, space="PSUM"))
        pvf = ctx.enter_context(tc.tile_pool(name="psvf", bufs=3, space="PSUM"))
        pools = (pbig, pgate, pchunk, pctx, pq, pvf, phv)

        def wload(name, dram, shape, dt):
            t = wp.tile(shape, dt, tag=name)
            nc.scalar.dma_start(t[:], dram[:])
            return t

        statW = wload("statW", statW_d, [128, KT, 3, 128], FP8)
        statH = wload("statH", statH_d, [128, KT, 3, 128], BF16)
        wkW = wload("wkW", wkW_d, [128, KT, 2, 128], BF16)
        wkH = wload("wkH", wkH_d, [128, KT, 2, 128], BF16)
        f2 = wload("f2", f2_d, [128, KT, 2, 128], BF16)
        fg = wload("fg", fg_d, [128, KT, 2, 128], BF16)

        bias_sb = wp.tile([128, 5, 2], F32, tag="biases")
        nc.scalar.dma_start(bias_sb[:], bias_d[:].transpose([2, 0, 1]))
        zb = wp.tile([128, 1], F32, tag="zb")
        nc.vector.memset(zb[:], 0.0)

        def bap(i, ct):
            return bias_sb[:, i, ct].unsqueeze(1)

        biasW = {
            "bv2": None if bvW0 else [bap(0, ct) for ct in range(2)],
            "bk2": None if bkW0 else [bap(1, ct) for ct in range(2)],
            "zb": zb[:],
        }
        biasH = {
            "bv2": None if bvH0 else [bap(2, ct) for ct in range(2)],
            "bk2": None if bkH0 else [bap(3, ct) for ct in range(2)],
            "zb": zb[:],
        }

        def load_x(b):
            # chunk-major, kt split across two DMA queues: first compute
            # chunk is ready after ~2 small transfers, not 9
            x = xp.tile([128, KT, HW], BF16, tag="x")
            for j in range(NCH):
                nc.sync.dma_start(x[:, 0, bass.ts(j, CH)], x_d[b, 0:128, bass.ts(j, CH)])
                nc.scalar.dma_start(x[:, 1, bass.ts(j, CH)], x_d[b, 128:256, bass.ts(j, CH)])
            return x

        def x8_loader(b):
            def load(j):
                x8 = x8p.tile([128, KT, CH], FP8, tag="x8")
                for kt in range(KT):
                    nc.scalar.dma_start(x8[:, kt, :], x8_d[b, bass.ts(kt, 128), bass.ts(j, CH)])
                return x8[:]
            return load

        def make_O(b):
            O_t = op_.tile([128, KT, HW], BF16, tag="O")
            return O_t

        # stage-W p3 pair: O = x + gate1*ctxb  (no matmul; feeds all H work)
        def w_p3_pair(st, x, O, jp):
            g2 = st.g2_pair(jp)
            g2f = g2[:].rearrange("p c a r -> p c (a r)")
            nc.vector.tensor_tensor(O[:, :, bass.ts(jp, CP)], x[:, :, bass.ts(jp, CP)], g2f, op=ALU.add)

        # stage-H p3: y = F2@O + FG@g2H per chunk, evict bf16
        def h_p3_chunk(st, O, g2p, b, j):
            ps_f = pvf.tile([128, 2 * CH], F32, tag="vf")
            sl = bass.ts(j, CH)
            ghalf = g2p[:, :, bass.ts(j % 2, GRP), :]
            for mt in range(2):
                half = ps_f[:, bass.ts(mt, CH)]
                nc.tensor.matmul(half, f2[:, 0, mt, :], O[:, 0, sl], start=True, stop=False)
                nc.tensor.matmul(half, f2[:, 1, mt, :], O[:, 1, sl], start=False, stop=False)
                nc.tensor.matmul(half, fg[:, 0, mt, :], ghalf[:, 0].rearrange("p a r -> p (a r)"), start=False, stop=False)
                nc.tensor.matmul(half, fg[:, 1, mt, :], ghalf[:, 1].rearrange("p a r -> p (a r)"), start=False, stop=True)
            y_t = yp.tile([128, 2, CH], BF16, tag="y")
            if byH0:
                nc.scalar.activation(y_t[:], ps_f[:].rearrange("p (c n) -> p c n", c=2), AF.Copy)
            else:
                for ct in range(2):
                    nc.scalar.activation(y_t[:, ct, :], ps_f[:, bass.ts(ct, CH)],
                                         AF.Identity, bias=bap(4, ct))
            nc.sync.dma_start(
                y_d[b].rearrange("(c p) n -> p c n", p=128)[:, :, sl], y_t[:])

        x0 = load_x(0)
        x1 = load_x(1)
        w0 = _Stage(nc, pools, True, x0[:], statW, wkW, biasW, src8=x8_loader(0))
        w1 = _Stage(nc, pools, True, x1[:], statW, wkW, biasW, src8=x8_loader(1))

        def run_p1(st):
            st.p1_alloc()
            for j in range(NCH):
                st.p1_chunk(j)
                if j % 2 == 1:
                    st.p1_pair(j // 2)

        run_p1(w0)
        run_p1(w1)
        w0.p2()

        O0 = make_O(0)
        h0 = _Stage(nc, pools, False, O0[:], statH, wkH, biasH)
        h0.p1_alloc()
        for jp in range(NP_):
            w_p3_pair(w0, x0, O0, jp)
            h0.p1_chunk(2 * jp)
            h0.p1_chunk(2 * jp + 1)
            h0.p1_pair(jp)

        w1.p2()
        O1 = make_O(1)
        h1 = _Stage(nc, pools, False, O1[:], statH, wkH, biasH)
        h1.p1_alloc()
        for jp in range(NP_):
            w_p3_pair(w1, x1, O1, jp)
            h1.p1_chunk(2 * jp)
            h1.p1_chunk(2 * jp + 1)
            h1.p1_pair(jp)

        h0.p2()
        for jp in range(NP_):
            g2p = h0.g2_pair(jp)
            h_p3_chunk(h0, O0, g2p, 0, 2 * jp)
            h_p3_chunk(h0, O0, g2p, 0, 2 * jp + 1)
        h1.p2()
        for jp in range(NP_):
            g2p = h1.g2_pair(jp)
            h_p3_chunk(h1, O1, g2p, 1, 2 * jp)
            h_p3_chunk(h1, O1, g2p, 1, 2 * jp + 1)

    nc.compile()
    return nc


def _to_stat(wq, wv):
    """[q replicated; v] -> lhsT layout [128, KT, 3, 128]."""
    stat = np.empty((128, KT, 3, 128), np.float32)
    for kt in range(KT):
        cs = slice(kt * 128, (kt + 1) * 128)
        stat[:, kt, 0, :] = wv[0:128, cs].T
        stat[:, kt, 1, :] = wv[128:256, cs].T
        stat[:, kt, 2, :] = np.repeat(wq[cs][:, None], 128, axis=1)
    return stat


def _to_lhsT(w):
    """[256out, 256in] -> [128, KT, 2, 128] (k-tile, m-tile)."""
    t = np.empty((128, KT, 2, 128), np.float32)
    for kt in range(KT):
        cs = slice(kt * 128, (kt + 1) * 128)
        t[:, kt, 0, :] = w[0:128, cs].T
        t[:, kt, 1, :] = w[128:256, cs].T
    return t


def kernel(x, qkvW_w, qkvW_b, qkvH_w, qkvH_b, fusW_w, fusW_b, fusH_w, fusH_b):
    global LAST_RESULTS
    x = np.asarray(x, np.float32)
    f64 = lambda a: np.asarray(a, np.float64)
    qkvW_w, qkvW_b = f64(qkvW_w), f64(qkvW_b)
    qkvH_w, qkvH_b = f64(qkvH_w), f64(qkvH_b)
    fusW_w, fusW_b = f64(fusW_w), f64(fusW_b)
    fusH_w, fusH_b = f64(fusH_w), f64(fusH_b)

    wqW, wkW_m, wvW = qkvW_w[0], qkvW_w[1:1 + C], qkvW_w[1 + C:]
    wqH, wkH_m, wvH = qkvH_w[0], qkvH_w[1:1 + C], qkvH_w[1 + C:]

    # collapse WfW into stage-H weights (x_w = WfW @ O + bfW)
    wqHp = wqH @ fusW_w
    wvHp = wvH @ fusW_w
    wkHp = wkH_m @ fusW_w
    F2 = fusH_w @ fusW_w

    statW = _to_stat(wqW.astype(np.float32), wvW.astype(np.float32))
    statH = _to_stat(wqHp.astype(np.float32), wvHp.astype(np.float32))
    wkWl = _to_lhsT(wkW_m.astype(np.float32))
    wkHl = _to_lhsT(wkHp.astype(np.float32))
    f2l = _to_lhsT(F2.astype(np.float32))
    fgl = _to_lhsT(fusH_w.astype(np.float32))

    # tanh-gate folding: ACT computes tanh(0.5*v + 0.5*bv); ctx is scaled by
    # 0.5 on device, so the ctx bias constant also carries the 0.5.
    bvW = qkvW_b[1 + C:]
    bkW = qkvW_b[1:1 + C]
    bvHp = wvH @ fusW_b + qkvH_b[1 + C:]
    bkHp = wkH_m @ fusW_b + qkvH_b[1:1 + C]   # sum_h softmax = 1 -> adds to ctx
    byH = fusH_w @ fusW_b + fusH_b
    biases = np.stack([
        0.5 * bvW.reshape(2, 128), 0.5 * bkW.reshape(2, 128),
        0.5 * bvHp.reshape(2, 128), 0.5 * bkHp.reshape(2, 128),
        byH.reshape(2, 128),
    ]).astype(np.float32)

    flags = (
        not bvW.any(), not bkW.any(), not bvHp.any(), not bkHp.any(), not byH.any(),
    )
    if flags not in _BUILD_CACHE:
        _BUILD_CACHE[flags] = _build(flags)
    nc = _BUILD_CACHE[flags]

    tobf = lambda a: np.ascontiguousarray(a.astype(NPBF))
    xbf = np.ascontiguousarray(x.reshape(B, C, HW).astype(NPBF))
    x8 = np.ascontiguousarray(x.reshape(B, C, HW).astype(NPF8))
    in_maps = []
    for core in range(NCORES):
        in_maps.append({
            "x": xbf[core * BPC: (core + 1) * BPC],
            "x8": x8[core * BPC: (core + 1) * BPC],
            "statW": np.ascontiguousarray(statW.astype(NPF8)),
            "statH": tobf(statH),
            "wkW": tobf(wkWl), "wkH": tobf(wkHl),
            "f2": tobf(f2l), "fg": tobf(fgl),
            "biases": biases,
        })

    res = run_bass_kernel_spmd(nc, in_maps, list(range(NCORES)))
    LAST_RESULTS = res
    y = np.concatenate([r["y"] for r in res.results], axis=0)
    return y.astype(np.float32).reshape(B, C, H, W)


# revision 14
# speedup vs baseline: 1.2364x; 1.2364x over previous
"""Trainium2 Bass kernel for nn_DualAxisAggAttn (dual-axis aggregation attention).

Reference semantics per batch image x[C=256, H=64, W=64], twice (W axis then H axis):
  qkv = conv1x1(x) -> {q:[1], k:[C], v:[C]};  s = softmax_axis(q)
  ctx[c,a] = sum_r k*s;  out = x + sigmoid(v) * ctx_bcast;  y = conv1x1(out)

Distribution: data-parallel over batch (16 images -> 2 per NeuronCore x 8 cores).

Structure (v4):
  - STAGE COLLAPSE: every stage-H op on x_w = WfW @ out_W is linear in
    channels, so WfW is folded into stage-H's weights on the HOST
    (qH' = qH@WfW, WvH' = WvH@WfW, WkH' = WkH@WfW, F2 = WfH@WfW). The
    stage-W fusion conv never runs on device; out_W is materialized once
    (O = x + g2) and feeds all H matmuls. Removes 2 of 8 big matmul
    passes AND the stage-W PSUM->SBUF eviction on the ACT engine.
  - stage-W qkv matmuls in fp8e4 DoubleRow (full 256-contraction per
    pass, 2x PE): only softmax weights E and the gate see the ~4%
    quantization, both of which are insensitive paths.
  - key-path linearity: ctx = Wk @ (sum_r x*E) / S (N=4096 -> 64).
  - q row replicated 128x in its m-tile -> exp(q) lands partition-broadcast.
  - sigmoid via tanh: {exp, tanh, copy} share ONE ACT table set
    (AF.Sigmoid does not -- a table swap costs 1.3us).
  - DVE has ~300ns/op overhead at ~0.38ns/elem/p streaming: elementwise
    ops (u, gate+1, g2, O) run on 1024-column chunk PAIRS; E lives in
    slot 0 of the [128, 3, HW] u tile so E+u reduce in ONE merged tree.
  - y stored bf16 on device, upcast to f32 on host (halves y DMA).
"""

import numpy as np
import ml_dtypes
from contextlib import ExitStack

import concourse.bass as bass
import concourse.bacc as bacc
import concourse.tile as tile
import concourse.mybir as mybir
from concourse.bass_utils import run_bass_kernel_spmd

F32 = mybir.dt.float32
BF16 = mybir.dt.bfloat16
FP8 = mybir.dt.float8e4
AF = mybir.ActivationFunctionType
ALU = mybir.AluOpType
AX = mybir.AxisListType
PM = mybir.MatmulPerfMode
NPBF = ml_dtypes.bfloat16
NPF8 = ml_dtypes.float8_e4m3fn

B, C, H, W = 16, 256, 64, 64
HW = H * W
NCORES = 8
BPC = B // NCORES
KT = 2
CH = 512
NCH = HW // CH
GRP = CH // 64
CP = 2 * CH          # DVE pair width
NP_ = HW // CP       # pairs per stage-batch

_BUILD_CACHE = {}
LAST_RESULTS = None


class _Stage:
    """One attention stage for one batch: p1 (qkv+u), p2 (reduce+ctx), p3."""

    def __init__(self, nc, pools, axis_w, src, stat, wk, bias, src8=None):
        self.nc, self.axis_w = nc, axis_w
        self.src, self.stat, self.wk, self.bias = src, stat, wk, bias
        self.src8 = src8  # callable j -> fp8 [128, KT, CH] chunk (W stage only)
        (self.pbig, self.pgate, self.pchunk, self.pctx, self.pq, self.pv,
         self.phv) = pools

    def p1_alloc(self):
        # slot 0 = E (exp(q)), slots 1:3 = u = x*E  -> one merged reduce tree
        self.u = self.pbig.tile([128, 3, HW], BF16, tag="u")
        self.gate = self.pgate.tile([128, 2, HW], BF16, tag="gate")

    def p1_chunk(self, j):
        nc, stat, bias = self.nc, self.stat, self.bias
        sl = bass.ts(j, CH)
        bv2 = bias.get("bv2")
        ps_q = self.pq.tile([128, CH], F32, tag="q")
        ps_v = self.pv.tile([128, 2 * CH], F32, tag="vf")
        if self.src8 is not None:
            x8 = self.src8(j)
            nc.tensor.matmul(ps_q[:], stat[:, :, 2, :], x8[:], perf_mode=PM.DoubleRow)
            nc.tensor.matmul(ps_v[:, 0:CH], stat[:, :, 0, :], x8[:], perf_mode=PM.DoubleRow)
            nc.tensor.matmul(ps_v[:, CH:], stat[:, :, 1, :], x8[:], perf_mode=PM.DoubleRow)
        else:
            for kt in range(KT):
                st, sp = kt == 0, kt == KT - 1
                rhs = self.src[:, kt, sl]
                nc.tensor.matmul(ps_q[:], stat[:, kt, 2, :], rhs, start=st, stop=sp)
                nc.tensor.matmul(ps_v[:, 0:CH], stat[:, kt, 0, :], rhs, start=st, stop=sp)
                nc.tensor.matmul(ps_v[:, CH:], stat[:, kt, 1, :], rhs, start=st, stop=sp)
        nc.scalar.activation(self.u[:, 0, sl], ps_q[:], AF.Exp, bias=bias["zb"])
        if bv2 is None:
            nc.scalar.activation(
                self.gate[:, :, sl], ps_v[:].rearrange("p (c n) -> p c n", c=2),
                AF.Tanh, scale=0.5,
            )
        else:
            nc.scalar.activation(self.gate[:, 0, sl], ps_v[:, 0:CH], AF.Tanh, bias=bv2[0], scale=0.5)
            nc.scalar.activation(self.gate[:, 1, sl], ps_v[:, CH:], AF.Tanh, bias=bv2[1], scale=0.5)

    def p1_pair(self, jp):
        """u = x*E and gate += 1 over a 1024-column chunk pair."""
        nc = self.nc
        slp = bass.ts(jp, CP)
        eb = self.u[:, 0, slp].unsqueeze(1).broadcast_to([128, 2, CP])
        nc.vector.tensor_tensor(self.u[:, 1:3, slp], self.src[:, :, slp], eb, op=ALU.mult)
        nc.vector.tensor_scalar_add(self.gate[:, :, slp], self.gate[:, :, slp], 1.0)

    def p2(self):
        nc, pctx, phv = self.nc, self.pctx, self.phv
        t3 = phv.tile([128, 3, 2048], BF16, tag="t3")
        SX = pctx.tile([128, 3, 64], F32, tag="SX")
        if self.axis_w:
            # reduce over w (inner 64): [3, 64 groups, r] halving + TR
            v4 = self.u[:].rearrange("p c (a r) -> p c a r", r=64)
            hv = t3[:].rearrange("p c (a r) -> p c a r", r=32)
            nc.vector.tensor_tensor(hv[:, :, :, :], v4[:, :, :, 0:32], v4[:, :, :, 32:64], op=ALU.add)
            nc.vector.tensor_tensor(hv[:, :, :, 0:16], hv[:, :, :, 0:16], hv[:, :, :, 16:32], op=ALU.add)
            nc.vector.tensor_tensor(hv[:, :, :, 0:8], hv[:, :, :, 0:8], hv[:, :, :, 8:16], op=ALU.add)
            nc.vector.tensor_reduce(SX[:], hv[:, :, :, 0:8], axis=AX.X, op=ALU.add)
        else:
            # reduce over h (outer): contiguous halving tree on [3, n]
            nc.vector.tensor_tensor(t3[:, :, :], self.u[:, :, 0:2048], self.u[:, :, 2048:4096], op=ALU.add)
            n = 1024
            while n >= 128:
                nc.vector.tensor_tensor(t3[:, :, 0:n], t3[:, :, 0:n], t3[:, :, n:2 * n], op=ALU.add)
                n //= 2
            nc.vector.tensor_tensor(SX[:], t3[:, :, 0:64], t3[:, :, 64:128], op=ALU.add)

        R = pctx.tile([128, 64], F32, tag="R")
        nc.vector.reciprocal(R[:], SX[:, 0, :])
        xn = pctx.tile([128, 2, 64], BF16, tag="xn")
        rb = R[:].unsqueeze(1).broadcast_to([128, 2, 64])
        nc.vector.tensor_tensor(xn[:], SX[:, 1:3, :], rb, op=ALU.mult)

        bk2 = self.bias.get("bk2")
        ctx_t = pctx.tile([128, 2, 64], BF16, tag="ctx")
        for mt in range(2):
            ps_c = self.pq.tile([128, 64], F32, tag="q")
            for ct in range(2):
                nc.tensor.matmul(ps_c[:], self.wk[:, ct, mt, :], xn[:, ct, :],
                                 start=ct == 0, stop=ct == 1)
            if bk2 is None:
                nc.vector.tensor_scalar_mul(ctx_t[:, mt, :], ps_c[:], 0.5)
            else:
                nc.vector.tensor_scalar(ctx_t[:, mt, :], ps_c[:], 0.5, bk2[mt],
                                        op0=ALU.mult, op1=ALU.add)
        self.ctx_t = ctx_t

    def g2_pair(self, jp):
        """g2 = gate1 * ctx_broadcast for a chunk pair -> [128, 2, 2*GRP, 64]."""
        nc = self.nc
        g2 = self.pchunk.tile([128, 2, 2 * GRP, 64], BF16, tag="g2")
        gv = self.gate[:, :, bass.ts(jp, CP)].rearrange("p c (a r) -> p c a r", r=64)
        if self.axis_w:
            cb = self.ctx_t[:, :, bass.ts(jp, 2 * GRP)].unsqueeze(3).broadcast_to([128, 2, 2 * GRP, 64])
        else:
            cb = self.ctx_t[:].unsqueeze(2).broadcast_to([128, 2, 2 * GRP, 64])
        nc.vector.tensor_tensor(g2[:], gv, cb, op=ALU.mult)
        return g2


def _build(flags):
    bvW0, bkW0, bvH0, bkH0, byH0 = flags
    nc = bacc.Bacc(trn_type="TRN2", target_bir_lowering=False, debug=False)

    x_d = nc.dram_tensor("x", [BPC, C, HW], BF16, kind="ExternalInput").ap()
    statW_d = nc.dram_tensor("statW", [128, KT, 3, 128], BF16, kind="ExternalInput").ap()
    statH_d = nc.dram_tensor("statH", [128, KT, 3, 128], BF16, kind="ExternalInput").ap()
    wkW_d = nc.dram_tensor("wkW", [128, KT, 2, 128], BF16, kind="ExternalInput").ap()
    wkH_d = nc.dram_tensor("wkH", [128, KT, 2, 128], BF16, kind="ExternalInput").ap()
    f2_d = nc.dram_tensor("f2", [128, KT, 2, 128], BF16, kind="ExternalInput").ap()
    fg_d = nc.dram_tensor("fg", [128, KT, 2, 128], BF16, kind="ExternalInput").ap()
    bias_d = nc.dram_tensor("biases", [5, 2, 128], F32, kind="ExternalInput").ap()
    y_d = nc.dram_tensor("y", [BPC, C, HW], BF16, kind="ExternalOutput").ap()

    with tile.TileContext(nc) as tc, ExitStack() as ctx:
        wp = ctx.enter_context(tc.tile_pool(name="weights", bufs=1))
        xp = ctx.enter_context(tc.tile_pool(name="x", bufs=2))
        op_ = ctx.enter_context(tc.tile_pool(name="O", bufs=2))
        pbig = ctx.enter_context(tc.tile_pool(name="big", bufs=2))
        pgate = ctx.enter_context(tc.tile_pool(name="gate", bufs=3))
        pchunk = ctx.enter_context(tc.tile_pool(name="chunk", bufs=2))
        pctx = ctx.enter_context(tc.tile_pool(name="ctx", bufs=3))
        phv = ctx.enter_context(tc.tile_pool(name="hv", bufs=1))
        yp = ctx.enter_context(tc.tile_pool(name="yev", bufs=2))
        pq = ctx.enter_context(tc.tile_pool(name="psq", bufs=2, space="PSUM"))
        pvf = ctx.enter_context(tc.tile_pool(name="psvf", bufs=3, space="PSUM"))
        pools = (pbig, pgate, pchunk, pctx, pq, pvf, phv)

        def wload(name, dram, shape, dt):
            t = wp.tile(shape, dt, tag=name)
            nc.scalar.dma_start(t[:], dram[:])
            return t

        statW = wload("statW", statW_d, [128, KT, 3, 128], BF16)
        statH = wload("statH", statH_d, [128, KT, 3, 128], BF16)
        wkW = wload("wkW", wkW_d, [128, KT, 2, 128], BF16)
        wkH = wload("wkH", wkH_d, [128, KT, 2, 128], BF16)
        f2 = wload("f2", f2_d, [128, KT, 2, 128], BF16)
        fg = wload("fg", fg_d, [128, KT, 2, 128], BF16)

        bias_sb = wp.tile([128, 5, 2], F32, tag="biases")
        nc.scalar.dma_start(bias_sb[:], bias_d[:].transpose([2, 0, 1]))
        zb = wp.tile([128, 1], F32, tag="zb")
        nc.vector.memset(zb[:], 0.0)

        def bap(i, ct):
            return bias_sb[:, i, ct].unsqueeze(1)

        biasW = {
            "bv2": None if bvW0 else [bap(0, ct) for ct in range(2)],
            "bk2": None if bkW0 else [bap(1, ct) for ct in range(2)],
            "zb": zb[:],
        }
        biasH = {
            "bv2": None if bvH0 else [bap(2, ct) for ct in range(2)],
            "bk2": None if bkH0 else [bap(3, ct) for ct in range(2)],
            "zb": zb[:],
        }

        def load_x(b):
            # chunk-major, kt split across two DMA queues: first compute
            # chunk is ready after ~2 small transfers, not 9
            x = xp.tile([128, KT, HW], BF16, tag="x")
            for j in range(NCH):
                for kt in range(KT):
                    nc.sync.dma_start(x[:, kt, bass.ts(j, CH)],
                                      x_d[b, bass.ts(kt, 128), bass.ts(j, CH)])
            return x

        def make_O(b):
            O_t = op_.tile([128, KT, HW], BF16, tag="O")
            return O_t

        # stage-W p3 pair: O = x + gate1*ctxb  (no matmul; feeds all H work)
        def w_p3_pair(st, x, O, jp):
            g2 = st.g2_pair(jp)
            g2f = g2[:].rearrange("p c a r -> p c (a r)")
            nc.vector.tensor_tensor(O[:, :, bass.ts(jp, CP)], x[:, :, bass.ts(jp, CP)], g2f, op=ALU.add)

        # stage-H p3: y = F2@O + FG@g2H per chunk, evict bf16
        def h_p3_chunk(st, O, g2p, b, j):
            ps_f = pvf.tile([128, 2 * CH], F32, tag="vf")
            sl = bass.ts(j, CH)
            ghalf = g2p[:, :, bass.ts(j % 2, GRP), :]
            for mt in range(2):
                half = ps_f[:, bass.ts(mt, CH)]
                nc.tensor.matmul(half, f2[:, 0, mt, :], O[:, 0, sl], start=True, stop=False)
                nc.tensor.matmul(half, f2[:, 1, mt, :], O[:, 1, sl], start=False, stop=False)
                nc.tensor.matmul(half, fg[:, 0, mt, :], ghalf[:, 0].rearrange("p a r -> p (a r)"), start=False, stop=False)
                nc.tensor.matmul(half, fg[:, 1, mt, :], ghalf[:, 1].rearrange("p a r -> p (a r)"), start=False, stop=True)
            y_t = yp.tile([128, 2, CH], BF16, tag="y")
            if byH0:
                nc.scalar.activation(y_t[:], ps_f[:].rearrange("p (c n) -> p c n", c=2), AF.Copy)
            else:
                for ct in range(2):
                    nc.scalar.activation(y_t[:, ct, :], ps_f[:, bass.ts(ct, CH)],
                                         AF.Identity, bias=bap(4, ct))
            nc.sync.dma_start(
                y_d[b].rearrange("(c p) n -> p c n", p=128)[:, :, sl], y_t[:])

        x0 = load_x(0)
        x1 = load_x(1)
        w0 = _Stage(nc, pools, True, x0[:], statW, wkW, biasW)
        w1 = _Stage(nc, pools, True, x1[:], statW, wkW, biasW)

        def run_p1(st):
            st.p1_alloc()
            for j in range(NCH):
                st.p1_chunk(j)
                if j % 2 == 1:
                    st.p1_pair(j // 2)

        run_p1(w0)
        run_p1(w1)
        w0.p2()

        O0 = make_O(0)
        h0 = _Stage(nc, pools, False, O0[:], statH, wkH, biasH)
        h0.p1_alloc()
        for jp in range(NP_):
            w_p3_pair(w0, x0, O0, jp)
            h0.p1_chunk(2 * jp)
            h0.p1_chunk(2 * jp + 1)
            h0.p1_pair(jp)

        w1.p2()
        O1 = make_O(1)
        h1 = _Stage(nc, pools, False, O1[:], statH, wkH, biasH)
        h1.p1_alloc()
        for jp in range(NP_):
            w_p3_pair(w1, x1, O1, jp)
            h1.p1_chunk(2 * jp)
            h1.p1_chunk(2 * jp + 1)
            h1.p1_pair(jp)

        h0.p2()
        for jp in range(NP_):
            g2p = h0.g2_pair(jp)
            h_p3_chunk(h0, O0, g2p, 0, 2 * jp)
            h_p3_chunk(h0, O0, g2p, 0, 2 * jp + 1)
        h1.p2()
        for jp in range(NP_):
            g2p = h1.g2_pair(jp)
            h_p3_chunk(h1, O1, g2p, 1, 2 * jp)
            h_p3_chunk(h1, O1, g2p, 1, 2 * jp + 1)

    nc.compile()
    return nc


def _to_stat(wq, wv):
    """[q replicated; v] -> lhsT layout [128, KT, 3, 128]."""
    stat = np.empty((128, KT, 3, 128), np.float32)
    for kt in range(KT):
        cs = slice(kt * 128, (kt + 1) * 128)
        stat[:, kt, 0, :] = wv[0:128, cs].T
        stat[:, kt, 1, :] = wv[128:256, cs].T
        stat[:, kt, 2, :] = np.repeat(wq[cs][:, None], 128, axis=1)
    return stat


def _to_lhsT(w):
    """[256out, 256in] -> [128, KT, 2, 128] (k-tile, m-tile)."""
    t = np.empty((128, KT, 2, 128), np.float32)
    for kt in range(KT):
        cs = slice(kt * 128, (kt + 1) * 128)
        t[:, kt, 0, :] = w[0:128, cs].T
        t[:, kt, 1, :] = w[128:256, cs].T
    return t


def kernel(x, qkvW_w, qkvW_b, qkvH_w, qkvH_b, fusW_w, fusW_b, fusH_w, fusH_b):
    global LAST_RESULTS
    x = np.asarray(x, np.float32)
    f64 = lambda a: np.asarray(a, np.float64)
    qkvW_w, qkvW_b = f64(qkvW_w), f64(qkvW_b)
    qkvH_w, qkvH_b = f64(qkvH_w), f64(qkvH_b)
    fusW_w, fusW_b = f64(fusW_w), f64(fusW_b)
    fusH_w, fusH_b = f64(fusH_w), f64(fusH_b)

    wqW, wkW_m, wvW = qkvW_w[0], qkvW_w[1:1 + C], qkvW_w[1 + C:]
    wqH, wkH_m, wvH = qkvH_w[0], qkvH_w[1:1 + C], qkvH_w[1 + C:]

    # collapse WfW into stage-H weights (x_w = WfW @ O + bfW)
    wqHp = wqH @ fusW_w
    wvHp = wvH @ fusW_w
    wkHp = wkH_m @ fusW_w
    F2 = fusH_w @ fusW_w

    statW = _to_stat(wqW.astype(np.float32), wvW.astype(np.float32))
    statH = _to_stat(wqHp.astype(np.float32), wvHp.astype(np.float32))
    wkWl = _to_lhsT(wkW_m.astype(np.float32))
    wkHl = _to_lhsT(wkHp.astype(np.float32))
    f2l = _to_lhsT(F2.astype(np.float32))
    fgl = _to_lhsT(fusH_w.astype(np.float32))

    # tanh-gate folding: ACT computes tanh(0.5*v + 0.5*bv); ctx is scaled by
    # 0.5 on device, so the ctx bias constant also carries the 0.5.
    bvW = qkvW_b[1 + C:]
    bkW = qkvW_b[1:1 + C]
    bvHp = wvH @ fusW_b + qkvH_b[1 + C:]
    bkHp = wkH_m @ fusW_b + qkvH_b[1:1 + C]   # sum_h softmax = 1 -> adds to ctx
    byH = fusH_w @ fusW_b + fusH_b
    biases = np.stack([
        0.5 * bvW.reshape(2, 128), 0.5 * bkW.reshape(2, 128),
        0.5 * bvHp.reshape(2, 128), 0.5 * bkHp.reshape(2, 128),
        byH.reshape(2, 128),
    ]).astype(np.float32)

    flags = (
        not bvW.any(), not bkW.any(), not bvHp.any(), not bkHp.any(), not byH.any(),
    )
    if flags not in _BUILD_CACHE:
        _BUILD_CACHE[flags] = _build(flags)
    nc = _BUILD_CACHE[flags]

    tobf = lambda a: np.ascontiguousarray(a.astype(NPBF))
    xbf = np.ascontiguousarray(x.reshape(B, C, HW).astype(NPBF))
    in_maps = []
    for core in range(NCORES):
        in_maps.append({
            "x": xbf[core * BPC: (core + 1) * BPC],
            "statW": tobf(statW), "statH": tobf(statH),
            "wkW": tobf(wkWl), "wkH": tobf(wkHl),
            "f2": tobf(f2l), "fg": tobf(fgl),
            "biases": biases,
        })

    res = run_bass_kernel_spmd(nc, in_maps, list(range(NCORES)))
    LAST_RESULTS = res
    y = np.concatenate([r["y"] for r in res.results], axis=0)
    return y.astype(np.float32).reshape(B, C, H, W)
